# revision 1
# baseline (speedup 1.0000x reference)
"""Trainium2 Bass kernel for nn_BasicNCAModel (neural cellular automaton).

Model (per step, 4 steps):
  y = concat([x, dwconv3x3(x, f1), dwconv3x3(x, f2)])   (reflect pad)
  dx = relu(y @ w1 + b1) @ w2
  x  = x + dx * (stoch > 0.5) * ch_mask

Kernel strategy:
  - Pure data parallel: batch 16 -> 2 samples on each of 8 NeuronCores.
  - Channel-major layout [C=32, H, W]; the depthwise convs + first dense
    layer fold into a single 3x3 conv with effective weights
    W_eff[dy,dx] = diag(f1[dy,dx]) @ w1[32:64] + diag(f2[dy,dx]) @ w1[64:96]
    (+ w1[0:32] at the center tap). Per 512-pixel tile this is 6 matmuls
    (3 horizontal taps x 2 output halves of 256) with K=96 = 3 vertically
    shifted copies of x stacked on partitions; horizontal taps are free-dim
    AP offsets. The bias rides as a 97th ones-row on the center tap.
    Second layer: 2 matmuls K=128. ch_mask is folded into w2 (cols 0..2
    zeroed), so the residual add is exact for the image channels.
  - Matmul operands are fp16 (full PE rate + fast weight load; ~2^-11
    rounding like fp32r but without its half-rate 2-pass behavior).
    The residual add x + dx*mask runs in exact fp32 on the vector engine
    from a separate fp32 load of the band interior.
  - x lives in DRAM column-padded [C, H, W+2] so band loads/stores are
    fully contiguous per partition; reflect rows are handled by DMA
    segmenting, reflect columns by two tiny on-chip copies. x ping-pongs
    between two internal DRAM buffers across the 4 steps.
"""

import numpy as np
from contextlib import ExitStack

import concourse.bacc as bacc
import concourse.tile as tile
from concourse import mybir
from concourse.bass_utils import run_bass_kernel_spmd

F32 = mybir.dt.float32
F16 = mybir.dt.float16
AF = mybir.ActivationFunctionType
OP = mybir.AluOpType

B, C, H, W = 16, 32, 256, 256
IMG = 3
FIRE = 0.5
NCORES = 8
BPC = B // NCORES          # samples per core = 2
BR = 16                    # band rows
NB = H // BR               # bands per sample = 16
ROWS_PER_TILE = 2          # 2 rows x 256 cols = 512-pixel matmul tiles
TPB = BR // ROWS_PER_TILE  # tiles per band = 8
NSTEP = 4
WP = W + 2                 # padded row length 258


def _seg_rows(r0: int, dy: int):
    """Contiguous (src_row, dst_row, n) segments for one vertical copy,
    with reflect handling at the image top/bottom (reflect: -1->1, 256->254)."""
    rows = [r0 + dy + i for i in range(BR)]
    refl = [(-r if r < 0 else (2 * (H - 1) - r if r > H - 1 else r)) for r in rows]
    segs = []
    i = 0
    while i < BR:
        j = i + 1
        while j < BR and refl[j] == refl[i] + (j - i):
            j += 1
        segs.append((refl[i], i, j - i))
        i = j
    return segs


def _build():
    nc = bacc.Bacc("TRN2", target_bir_lowering=False, debug=False,
                   num_devices=NCORES)
    xin = nc.dram_tensor("xin", [BPC, C, H, WP], F32, kind="ExternalInput").ap()
    stoch = nc.dram_tensor("stoch", [NSTEP, BPC, H, W], F32,
                           kind="ExternalInput").ap()
    wm = nc.dram_tensor("wm", [96, 256], F16, kind="ExternalInput").ap()
    w0 = nc.dram_tensor("w0", [97, 256], F16, kind="ExternalInput").ap()
    wp = nc.dram_tensor("wp", [96, 256], F16, kind="ExternalInput").ap()
    w2h = nc.dram_tensor("w2h", [128, 64], F16, kind="ExternalInput").ap()
    yout = nc.dram_tensor("y", [BPC, C, H, WP], F32, kind="ExternalOutput").ap()

    with tile.TileContext(nc) as tc, ExitStack() as ctx:
        dram = ctx.enter_context(tc.tile_pool(name="dram", bufs=1, space="DRAM"))
        xA = dram.tile([BPC, C, H, WP], F32, name="xA")
        xB = dram.tile([BPC, C, H, WP], F32, name="xB")

        wpool = ctx.enter_context(tc.tile_pool(name="wpool", bufs=1))
        wmt = wpool.tile([96, 256], F16, name="wmt")
        w0t = wpool.tile([97, 256], F16, name="w0t")
        wpt = wpool.tile([96, 256], F16, name="wpt")
        w2t = wpool.tile([128, 64], F16, name="w2t")
        ones = wpool.tile([1, BR * WP], F16, name="ones")
        nc.sync.dma_start(wmt[:], wm)
        nc.sync.dma_start(w0t[:], w0)
        nc.sync.dma_start(wpt[:], wp)
        nc.sync.dma_start(w2t[:], w2h)
        nc.gpsimd.memset(ones[:], 1.0)

        xt_pool = ctx.enter_context(tc.tile_pool(name="xt", bufs=4))
        xc_pool = ctx.enter_context(tc.tile_pool(name="xc", bufs=2))
        st_pool = ctx.enter_context(tc.tile_pool(name="st", bufs=2))
        stb_pool = ctx.enter_context(tc.tile_pool(name="stb", bufs=2))
        xn_pool = ctx.enter_context(tc.tile_pool(name="xn", bufs=2))
        hs_pool = ctx.enter_context(tc.tile_pool(name="hs", bufs=3))
        dxm_pool = ctx.enter_context(tc.tile_pool(name="dxm", bufs=3))
        hp_pool = ctx.enter_context(tc.tile_pool(name="hp", bufs=3, space="PSUM"))
        dxp_pool = ctx.enter_context(tc.tile_pool(name="dxp", bufs=2, space="PSUM"))

        srcs = [xin, xA[:], xB[:], xA[:]]
        dsts = [xA[:], xB[:], xA[:], yout]

        for step in range(NSTEP):
            src, dst = srcs[step], dsts[step]
            for s in range(BPC):
                for b in range(NB):
                    r0 = b * BR
                    # ---- load: 3 vertically shifted fp16 copies of the band.
                    # partition groups: 0-31 dy=0 (center), 32-63 dy=-1,
                    # 64-95 dy=+1 — center first so the residual/mask ops all
                    # share base partition 0 (DVE needs equal base partitions).
                    xt = xt_pool.tile([97, BR * WP], F16)
                    xtr = xt[:].rearrange("p (r c) -> p r c", c=WP)
                    for gi, dy in enumerate((0, -1, 1)):
                        p0 = gi * 32
                        for (sr, dr, n) in _seg_rows(r0, dy):
                            # SWDGE load, cast f32 -> fp16 in flight (cheap
                            # trigger; descriptor gen runs on Q7 cores, off
                            # the engine queues); contiguous [n*WP]/channel
                            nc.gpsimd.dma_start(
                                xtr[p0:p0 + 32, dr:dr + n, :],
                                src[s, :, sr:sr + n, :])
                    # ones row for the bias (97th K row of the center tap)
                    nc.gpsimd.dma_start(xt[96:97, :], ones[:])
                    # reflect column pads: col0 <- col2, col257 <- col255
                    nc.vector.tensor_copy(xtr[0:96, :, 0:1], xtr[0:96, :, 2:3])
                    nc.vector.tensor_copy(xtr[0:96, :, WP - 1:WP],
                                          xtr[0:96, :, WP - 3:WP - 2])

                    # exact fp32 copy of the band for the residual add
                    xc = xc_pool.tile([32, BR * WP], F32)
                    xcr = xc[:].rearrange("p (r c) -> p r c", c=WP)
                    nc.gpsimd.dma_start(xc[:], src[s, :, r0:r0 + BR, :]
                                        .rearrange("p r c -> p (r c)"))

                    # ---- stochastic values, broadcast across channels ----
                    st = st_pool.tile([1, BR * W], F32)
                    nc.gpsimd.dma_start(
                        st[:], stoch[step, s, r0:r0 + BR, :].flatten().unsqueeze(0))
                    stb = stb_pool.tile([32, BR * W], F32)
                    nc.gpsimd.partition_broadcast(stb[:], st[:])
                    stbr = stb[:].rearrange("p (r c) -> p r c", c=W)

                    xn = xn_pool.tile([32, BR * WP], F32)
                    xnr = xn[:].rearrange("p (r c) -> p r c", c=WP)
                    # pad columns are stored to DRAM but never consumed as
                    # data; init them so the contiguous store reads defined
                    # memory (single strided memset covers cols 0 and 257)
                    nc.gpsimd.memset(xnr[:, :, 0:WP:WP - 1], 0.0)

                    # software pipeline: layer 2 of tile t-1 is emitted after
                    # layer 1 of tile t, giving the relu a full L1-block of
                    # slack before the PE needs its output
                    pend = None  # (hs, rt) awaiting layer 2
                    for t in range(TPB + 1):
                        if t < TPB:
                            rt = t * ROWS_PER_TILE
                            # ---- layer 1: 3x3 conv (3 taps x 2 halves) ----
                            hp = hp_pool.tile([128, 1024], F32)
                            for h in range(2):
                                out = hp[:, h * 512:(h + 1) * 512]
                                hslc = slice(h * 128, (h + 1) * 128)
                                nc.tensor.matmul(
                                    out, wmt[:, hslc],
                                    xtr[0:96, rt:rt + ROWS_PER_TILE, 0:W],
                                    start=True, stop=False)
                                nc.tensor.matmul(
                                    out, w0t[:, hslc],
                                    xtr[0:97, rt:rt + ROWS_PER_TILE, 1:W + 1],
                                    start=False, stop=False)
                                nc.tensor.matmul(
                                    out, wpt[:, hslc],
                                    xtr[0:96, rt:rt + ROWS_PER_TILE, 2:W + 2],
                                    start=False, stop=True)
                            # ---- relu (bias already added via ones row) ----
                            hs = hs_pool.tile([128, 1024], F16)
                            nc.scalar.activation(hs[:], hp[:], AF.Relu)
                            pend, prev = (hs, rt), pend
                        else:
                            prev, pend = pend, None
                        if prev is None:
                            continue
                        hs_p, rp = prev
                        # ---- layer 2: dx = h @ w2 (K=256 split in two) ----
                        dxp = dxp_pool.tile([32, 512], F32)
                        nc.tensor.matmul(dxp[:], w2t[:, 0:32], hs_p[:, 0:512],
                                         start=True, stop=False)
                        nc.tensor.matmul(dxp[:], w2t[:, 32:64], hs_p[:, 512:1024],
                                         start=False, stop=True)
                        # ---- masked residual: xn = (stoch>0.5)*dx + x ----
                        dxpr = dxp[:].rearrange("p (r c) -> p r c", c=W)
                        dxm = dxm_pool.tile([32, ROWS_PER_TILE * W], F32)
                        dxmr = dxm[:].rearrange("p (r c) -> p r c", c=W)
                        nc.vector.scalar_tensor_tensor(
                            dxmr, stbr[:, rp:rp + ROWS_PER_TILE, :], FIRE,
                            dxpr, op0=OP.is_gt, op1=OP.mult)
                        nc.vector.tensor_add(
                            xnr[:, rp:rp + ROWS_PER_TILE, 1:W + 1], dxmr,
                            xcr[:, rp:rp + ROWS_PER_TILE, 1:W + 1])

                    # ---- store band (contiguous, pads included) ----
                    nc.sync.dma_start(
                        dst[s, :, r0:r0 + BR, :].rearrange("p r c -> p (r c)"),
                        xn[:])
    nc.compile()
    return nc


_NC_CACHE = None


def _get_nc():
    global _NC_CACHE
    if _NC_CACHE is None:
        _NC_CACHE = _build()
    return _NC_CACHE


def _make_in_maps(x, f1, f2, w1, b1, w2, stoch):
    f1 = np.asarray(f1, np.float64)[:, :, 0, :]   # [3,3,32]
    f2 = np.asarray(f2, np.float64)[:, :, 0, :]
    w1 = np.asarray(w1, np.float64)               # [96,256]
    b1 = np.asarray(b1, np.float64)               # [256]
    w2 = np.asarray(w2, np.float64).copy()        # [256,32]
    w2[:, :IMG] = 0.0                             # ch_mask folded into w2

    # W_eff[dy,dx][c,:] = f1[dy,dx,c]*w1[32+c,:] + f2[dy,dx,c]*w1[64+c,:]
    #                     (+ w1[c,:] at the center tap)
    weff = (f1[:, :, :, None] * w1[None, None, 32:64, :]
            + f2[:, :, :, None] * w1[None, None, 64:96, :])   # [3,3,32,256]
    weff[1, 1] += w1[0:32, :]

    def col(dxi):  # stack the 3 vertical taps along K for horizontal tap dxi
        # row order matches xt partition groups: dy=0, dy=-1, dy=+1
        return np.concatenate([weff[1, dxi], weff[0, dxi], weff[2, dxi]], axis=0)

    wm = col(0).astype(np.float16)                                    # [96,256]
    w0 = np.concatenate([col(1), b1[None, :]], axis=0).astype(np.float16)
    wpm = col(2).astype(np.float16)                                   # [96,256]
    w2h = np.concatenate([w2[0:128, :], w2[128:256, :]], axis=1).astype(np.float16)

    x = np.asarray(x, np.float32)
    stoch = np.asarray(stoch, np.float32)
    in_maps = []
    for i in range(NCORES):
        xi = np.transpose(x[i * BPC:(i + 1) * BPC], (0, 3, 1, 2))  # [2,32,H,W]
        xpad = np.zeros((BPC, C, H, WP), np.float32)
        xpad[:, :, :, 1:W + 1] = xi
        sti = np.ascontiguousarray(
            stoch[:, i * BPC:(i + 1) * BPC, :, :, 0])
        in_maps.append({"xin": xpad, "stoch": sti, "wm": wm, "w0": w0,
                        "wp": wpm, "w2h": w2h})
    return in_maps


def kernel(x, f1, f2, w1, b1, w2, stoch, steps):
    assert int(steps) == NSTEP, f"kernel compiled for {NSTEP} steps, got {steps}"
    nc = _get_nc()
    in_maps = _make_in_maps(x, f1, f2, w1, b1, w2, stoch)
    res = run_bass_kernel_spmd(nc, in_maps, core_ids=list(range(NCORES)))
    outs = []
    for i in range(NCORES):
        yi = res.results[i]["y"][:, :, :, 1:W + 1]     # strip col pads
        outs.append(np.transpose(yi, (0, 2, 3, 1)))    # -> [2,256,256,32]
    return np.ascontiguousarray(np.concatenate(outs, axis=0)).astype(np.float32)



# revision 6
# speedup vs baseline: 1.0248x; 1.0248x over previous
"""Trainium2 Bass kernel for nn_BasicNCAModel (neural cellular automaton).

Model (per step, 4 steps):
  y = concat([x, dwconv3x3(x, f1), dwconv3x3(x, f2)])   (reflect pad)
  dx = relu(y @ w1 + b1) @ w2
  x  = x + dx * (stoch > 0.5) * ch_mask

Strategy (v2, fp8-DoubleRow hybrid):
  - Pure data parallel: batch 16 -> 2 samples per core, samples sequential.
  - x lives RESIDENT in SBUF in fp16 ("xex", layout [128, (H/4)*W]:
    partition = 32*(row%4) + channel), updated in place each step. No DRAM
    round-trips between steps; DRAM I/O is the initial load, the per-step
    masks, and the final store.
  - The depthwise convs + first dense layer fold into a 3x3 conv with
    effective weights W_eff[dy,dx] (the baseline trick). Per output row
    (256 px) the conv splits into:
      * chunk A (fp8 e4m3, DoubleRow): the 6 off-center-column taps
        (dy in {-1,0,1} x dx in {0,2}) packed 2-per-PE-cell via an
        overlapping moving AP [97,(2,stride 2),(1,256)], + bias via a
        ones partition. 1 matmul per N-half.
      * chunk B (fp16): the 3 center-column taps (dx=1) - these carry the
        dominant identity path w1[0:32], so they stay in fp16 for accuracy.
        1 matmul per N-half.
    Layer 2 runs in fp16 (2 matmuls, K=128 each). Total 6 matmuls / row
    vs 8 fp16-only: matmul cost on TRN2 is out_rows * 0.42ns regardless
    of K<=128 (2*128 with DoubleRow), so this is a 1.33x tensor win with
    fp8 confined to the small off-center taps (max rel err ~1.3e-2 vs
    2e-2 budget, numpy-simulated).
  - Per 16-row band, shifted views of x are STAGED from xex:
    stage16 [96, 16*256] fp16 (3 dy-shifted copies, HWDGE) for chunk B,
    stage8 [97, 16*258] fp8 (SWDGE cast, + reflect col pads) for chunk A.
    Stage loads for band b+1 are emitted before band b's residual writes,
    so the in-place xex update never corrupts halo reads.
  - Residual: dx masked with host-precomputed {0,1} fp8 masks (replicated
    to the [128, .] layout), added to xex in fp32->fp16 on the vector
    engine, 128 partitions wide (4 rows per op).
"""

import numpy as np
import ml_dtypes
from contextlib import ExitStack

import concourse.bacc as bacc
import concourse.tile as tile
from concourse import mybir
from concourse.bass_utils import run_bass_kernel_spmd
import bass_rust

F32 = mybir.dt.float32
F16 = mybir.dt.float16
F8 = mybir.dt.float8e4
AF = mybir.ActivationFunctionType
OP = mybir.AluOpType
DR = mybir.MatmulPerfMode.DoubleRow

B, C, H, W = 16, 32, 256, 256
IMG = 3
NCORES = 8
BPC = B // NCORES          # samples per core = 2
NSTEP = 4
WP = W + 2                 # stage8 row pitch (reflect col pads)
BR = 16                    # band rows
NB = H // BR               # bands = 16
GQ = H // 4                # 4-row groups per image = 64
DYS = (0, -1, 1)           # dy per stage partition group


def _refl(r):
    return -r if r < 0 else (2 * (H - 1) - r if r > H - 1 else r)


def _mv_dr(t, npart, m, i_stride, off):
    """Overlapping DoubleRow moving AP [npart, (2,i_stride), (1,m)]."""
    ap = t[0:npart, off:off + 1].unsqueeze(1)
    fstride = t[:].ap[0][0]
    ap.ap = bass_rust.VecI64Pair([(fstride, npart), (i_stride, 2), (1, m)])
    return ap


def _build():
    nc = bacc.Bacc("TRN2", target_bir_lowering=False, debug=False,
                   num_devices=NCORES)
    xin = nc.dram_tensor("xin", [BPC, C, H, W], F32, kind="ExternalInput").ap()
    msk = nc.dram_tensor("msk", [NSTEP, BPC, 128, GQ * W], F8,
                         kind="ExternalInput").ap()
    wa = nc.dram_tensor("wa", [97, 512], F8, kind="ExternalInput").ap()
    wb = nc.dram_tensor("wb", [96, 256], F16, kind="ExternalInput").ap()
    w2 = nc.dram_tensor("w2", [128, 64], F16, kind="ExternalInput").ap()
    yout = nc.dram_tensor("y", [BPC, 128, GQ * W], F16, kind="ExternalOutput").ap()

    with tile.TileContext(nc) as tc, ExitStack() as ctx:
        wpool = ctx.enter_context(tc.tile_pool(name="wpool", bufs=1))
        wat = wpool.tile([97, 512], F8, name="wat")
        wbt = wpool.tile([96, 256], F16, name="wbt")
        w2t = wpool.tile([128, 64], F16, name="w2t")
        nc.sync.dma_start(wat[:], wa)
        nc.sync.dma_start(wbt[:], wb)
        nc.sync.dma_start(w2t[:], w2)
        war = wat[:].rearrange("p (i n) -> p i n", i=2)   # [97, 2, 256]

        xex = wpool.tile([128, GQ * W], F16, name="xex")
        # stage tiles: 2 alternating buffers each (band parity), persistent
        # so the stage8 ones-row (bias) survives across bands.
        st16 = [wpool.tile([96, BR * W], F16, name=f"st16_{i}") for i in (0, 1)]
        st8 = [wpool.tile([97, BR * WP], F8, name=f"st8_{i}") for i in (0, 1)]
        for t in st8:
            nc.gpsimd.memset(t[96:97, :], 1.0)

        mk_pool = ctx.enter_context(tc.tile_pool(name="mk", bufs=2))
        hs_pool = ctx.enter_context(tc.tile_pool(name="hs", bufs=6))
        t_pool = ctx.enter_context(tc.tile_pool(name="tp", bufs=2))
        hp_pool = ctx.enter_context(tc.tile_pool(name="hp", bufs=6, space="PSUM"))
        dx_pool = ctx.enter_context(tc.tile_pool(name="dxp", bufs=2, space="PSUM"))

        def load_stage(s, b):
            """Stage band b's shifted x views from xex (reads pre-update
            values: call before band b-1's residual writes land)."""
            s16, s8 = st16[b % 2], st8[b % 2]
            s16r = s16[:].rearrange("p (r c) -> p r c", c=W)
            s8r = s8[:].rearrange("p (r c) -> p r c", c=WP)
            xr = xex[:].rearrange("p (g c) -> p g c", c=W)
            for gi, dy in enumerate(DYS):
                p0 = gi * 32
                # stage slot rr holds x row refl(16b+rr+dy)
                rows = [_refl(BR * b + rr + dy) for rr in range(BR)]
                # batch by phase where rows form stride-4 runs
                done = [False] * BR
                for rr0 in range(BR):
                    if done[rr0]:
                        continue
                    run = [rr0]
                    while len(run) < 4:
                        nxt = run[-1] + 4
                        if nxt < BR and rows[nxt] == rows[run[-1]] + 4:
                            run.append(nxt)
                        else:
                            break
                    for rr in run:
                        done[rr] = True
                    r0 = rows[rr0]
                    ph, g0, n = r0 % 4, r0 // 4, len(run)
                    src = xr[32 * ph:32 * ph + 32, g0:g0 + n, :]
                    end = rr0 + 4 * (n - 1) + 1
                    nc.sync.dma_start(
                        s16r[p0:p0 + 32, rr0:end:4, :], src)
                    nc.gpsimd.dma_start(
                        s8r[p0:p0 + 32, rr0:end:4, 1:W + 1], src)
            # reflect col pads for stage8 (cols 0 and 257)
            nc.vector.tensor_copy(s8r[0:96, :, 0:1], s8r[0:96, :, 2:3])
            nc.vector.tensor_copy(s8r[0:96, :, WP - 1:WP],
                                  s8r[0:96, :, WP - 3:WP - 2])
            return s16, s8

        def do_l2(nc2, item):
            """Layer 2 for one row; on the group's last row, emit the
            masked-residual chain for that group's dxp."""
            hs, j, dxp, mk, g4, g = item
            out = dxp[32 * j:32 * j + 32, 0:256]
            nc2.tensor.matmul(out, w2t[:, 0:32], hs[:, 0:256],
                              start=True, stop=False, tile_position=(0, 32 * j))
            nc2.tensor.matmul(out, w2t[:, 32:64], hs[:, 256:512],
                              start=False, stop=True, tile_position=(0, 32 * j))
            if j == 3:
                # ---- masked residual: xex += dxp * mask ----
                tt = t_pool.tile([128, 256], F32)
                nc2.vector.tensor_tensor(
                    tt[:], dxp[:, 0:256],
                    mk[:, g4 * 256:(g4 + 1) * 256], OP.mult)
                nc2.vector.tensor_tensor(
                    xex[:, g * W:(g + 1) * W], tt[:],
                    xex[:, g * W:(g + 1) * W], OP.add)

        for s in range(BPC):
            # ---- init: load x into xex (fp32 -> fp16, 4 phase DMAs) ----
            xr = xex[:].rearrange("p (g c) -> p g c", c=W)
            for ph in range(4):
                nc.gpsimd.dma_start(
                    xr[32 * ph:32 * ph + 32, :, :],
                    xin[s, :, ph:H:4, :])
            for st in range(NSTEP):
                pend = []  # rows awaiting layer 2 (software pipeline, lag 2)
                for b in range(NB):
                    if b == 0:
                        s16, s8 = load_stage(s, 0)
                        mk = mk_pool.tile([128, 1024], F8)
                        nc.sync.dma_start(mk[:], msk[st, s, :, 0:1024])
                    else:
                        s16, s8 = st16[b % 2], st8[b % 2]
                        mk = mk_nxt
                    # prefetch next band's stages + mask BEFORE this band's
                    # residual writes touch xex (halo correctness)
                    if b + 1 < NB:
                        load_stage(s, b + 1)
                        mk_nxt = mk_pool.tile([128, 1024], F8)
                        nc.sync.dma_start(
                            mk_nxt[:],
                            msk[st, s, :, (b + 1) * 1024:(b + 2) * 1024])
                    s16f = s16[:]
                    for g4 in range(4):
                        dxp = dx_pool.tile([128, 512], F32)
                        for j in range(4):
                            rr = 4 * g4 + j
                            hp = hp_pool.tile([128, 512], F32)
                            mva = _mv_dr(s8, 97, 256, 2, rr * WP)
                            mvb = s16f[0:96, rr * W:(rr + 1) * W]
                            # one accumulation group per PSUM bank: A(h0)'s
                            # start zeroes the 2KB region, A(h1) lands on
                            # pending-zero bytes, B's accumulate on top.
                            nc.tensor.matmul(hp[:, 0:256], war[:, :, 0:128],
                                             mva, start=True, stop=False,
                                             perf_mode=DR)
                            nc.tensor.matmul(hp[:, 256:512], war[:, :, 128:256],
                                             mva, start=False, stop=False,
                                             perf_mode=DR)
                            nc.tensor.matmul(hp[:, 0:256], wbt[:, 0:128],
                                             mvb, start=False, stop=False)
                            nc.tensor.matmul(hp[:, 256:512], wbt[:, 128:256],
                                             mvb, start=False, stop=True)
                            hs = hs_pool.tile([128, 512], F16)
                            if rr % 2 == 0:
                                nc.scalar.activation(hs[:], hp[:], AF.Relu)
                            else:
                                nc.vector.tensor_relu(hs[:], hp[:])
                            pend.append((hs, j, dxp, mk, g4, 4 * b + g4))
                            if len(pend) > 2:
                                do_l2(nc, pend.pop(0))
                while pend:
                    do_l2(nc, pend.pop(0))
            # ---- store sample result (raw xex layout, host unpacks) ----
            nc.sync.dma_start(yout[s], xex[:])
    nc.compile()
    return nc


_NC_CACHE = None


def _get_nc():
    global _NC_CACHE
    if _NC_CACHE is None:
        _NC_CACHE = _build()
    return _NC_CACHE


def _make_in_maps(x, f1, f2, w1, b1, w2, stoch):
    F8N = ml_dtypes.float8_e4m3
    f1 = np.asarray(f1, np.float64)[:, :, 0, :]   # [3,3,32]
    f2 = np.asarray(f2, np.float64)[:, :, 0, :]
    w1 = np.asarray(w1, np.float64)               # [96,256]
    b1 = np.asarray(b1, np.float64)               # [256]
    w2m = np.asarray(w2, np.float64).copy()       # [256,32]
    w2m[:, :IMG] = 0.0

    # W_eff[dy,dx][c,:] = f1*w1[32:64] + f2*w1[64:96] (+ w1[0:32] center)
    weff = (f1[:, :, :, None] * w1[None, None, 32:64, :]
            + f2[:, :, :, None] * w1[None, None, 64:96, :])   # [3,3,32,256]
    weff[1, 1] += w1[0:32, :]

    # chunk A (fp8 DR): taps (dy, dx in {0,2}); p = dyg*32+c, dyg=(0,-1,+1)
    dy_rows = (1, 0, 2)  # weff dy index per partition group
    wa = np.zeros((97, 2, 256), np.float64)
    for gi, dyi in enumerate(dy_rows):
        for i, dxi in enumerate((0, 2)):
            wa[gi * 32:(gi + 1) * 32, i, :] = weff[dyi, dxi]
    wa[96, 0, :] = b1
    wa8 = wa.astype(F8N).reshape(97, 512)
    # chunk B (fp16): center column taps (dy, dx=1)
    wbm = np.zeros((96, 256), np.float64)
    for gi, dyi in enumerate(dy_rows):
        wbm[gi * 32:(gi + 1) * 32, :] = weff[dyi, 1]
    wb16 = wbm.astype(np.float16)
    # layer 2 (fp16): [p, h*32+n] = w2m[h*128+p, n]
    w2h = np.concatenate([w2m[0:128, :], w2m[128:256, :]],
                         axis=1).astype(np.float16)

    x = np.asarray(x, np.float32)
    stoch = np.asarray(stoch, np.float32)
    in_maps = []
    for i in range(NCORES):
        xi = np.ascontiguousarray(
            np.transpose(x[i * BPC:(i + 1) * BPC], (0, 3, 1, 2)))  # [2,32,H,W]
        m = (stoch[:, i * BPC:(i + 1) * BPC, :, :, 0] > 0.5)  # [4,2,H,W]
        # [st,s,128,GQ*W]: partition 32*ph+c <- mask row 4g+ph
        mm = m.reshape(NSTEP, BPC, GQ, 4, W).transpose(0, 1, 3, 2, 4)
        mm = mm.reshape(NSTEP, BPC, 4, GQ * W)
        mrep = np.repeat(mm, 32, axis=2).astype(F8N)
        in_maps.append({"xin": xi, "msk": mrep, "wa": wa8, "wb": wb16,
                        "w2": w2h})
    return in_maps


def kernel(x, f1, f2, w1, b1, w2, stoch, steps):
    assert int(steps) == NSTEP, f"kernel compiled for {NSTEP} steps, got {steps}"
    nc = _get_nc()
    in_maps = _make_in_maps(x, f1, f2, w1, b1, w2, stoch)
    res = run_bass_kernel_spmd(nc, in_maps, core_ids=list(range(NCORES)))
    outs = []
    for i in range(NCORES):
        yi = np.asarray(res.results[i]["y"], np.float32)  # [2,128,GQ*W]
        # partition 32*ph+c, free g*W+col -> x[c, 4g+ph, col]
        yi = yi.reshape(BPC, 4, 32, GQ, W).transpose(0, 2, 3, 1, 4)
        outs.append(yi.reshape(BPC, C, H, W).transpose(0, 2, 3, 1))
    return np.ascontiguousarray(np.concatenate(outs, axis=0)).astype(np.float32)


# revision 12
# speedup vs baseline: 1.1449x; 1.1172x over previous
"""Trainium2 Bass kernel for nn_BasicNCAModel (neural cellular automaton).

Model (per step, 4 steps):
  y = concat([x, dwconv3x3(x, f1), dwconv3x3(x, f2)])   (reflect pad)
  dx = relu(y @ w1 + b1) @ w2
  x  = x + dx * (stoch > 0.5) * ch_mask

Strategy (v2, fp8-DoubleRow hybrid):
  - Pure data parallel: batch 16 -> 2 samples per core, samples sequential.
  - x lives RESIDENT in SBUF in fp16 ("xex", layout [128, (H/4)*W]:
    partition = 32*(row%4) + channel), updated in place each step. No DRAM
    round-trips between steps; DRAM I/O is the initial load, the per-step
    masks, and the final store.
  - The depthwise convs + first dense layer fold into a 3x3 conv with
    effective weights W_eff[dy,dx] (the baseline trick). Per output row
    (256 px) the conv splits into:
      * chunk A (fp8 e4m3, DoubleRow): the 6 off-center-column taps
        (dy in {-1,0,1} x dx in {0,2}) packed 2-per-PE-cell via an
        overlapping moving AP [97,(2,stride 2),(1,256)], + bias via a
        ones partition. 1 matmul per N-half.
      * chunk B (fp16): the 3 center-column taps (dx=1) - these carry the
        dominant identity path w1[0:32], so they stay in fp16 for accuracy.
        1 matmul per N-half.
    Layer 2 runs in fp16 (2 matmuls, K=128 each). Total 6 matmuls / row
    vs 8 fp16-only: matmul cost on TRN2 is out_rows * 0.42ns regardless
    of K<=128 (2*128 with DoubleRow), so this is a 1.33x tensor win with
    fp8 confined to the small off-center taps (max rel err ~1.3e-2 vs
    2e-2 budget, numpy-simulated).
  - Per 16-row band, shifted views of x are STAGED from xex:
    stage16 [96, 16*256] fp16 (3 dy-shifted copies, HWDGE) for chunk B,
    stage8 [97, 16*258] fp8 (SWDGE cast, + reflect col pads) for chunk A.
    Stage loads for band b+1 are emitted before band b's residual writes,
    so the in-place xex update never corrupts halo reads.
  - Residual: dx masked with host-precomputed {0,1} fp8 masks (replicated
    to the [128, .] layout), added to xex in fp32->fp16 on the vector
    engine, 128 partitions wide (4 rows per op).
"""

import numpy as np
import ml_dtypes
from contextlib import ExitStack

import concourse.bacc as bacc
import concourse.tile as tile
from concourse import mybir
from concourse.bass_utils import run_bass_kernel_spmd
import bass_rust

F32 = mybir.dt.float32
F16 = mybir.dt.float16
F8 = mybir.dt.float8e4
AF = mybir.ActivationFunctionType
OP = mybir.AluOpType
DR = mybir.MatmulPerfMode.DoubleRow

B, C, H, W = 16, 32, 256, 256
IMG = 3
NCORES = 8
BPC = B // NCORES          # samples per core = 2
NSTEP = 4
WP = W + 2                 # stage8 row pitch (reflect col pads)
BR = 16                    # band rows
NB = H // BR               # bands = 16
GQ = H // 4                # 4-row groups per image = 64
DYS = (0, -1, 1)           # dy per stage partition group


def _refl(r):
    return -r if r < 0 else (2 * (H - 1) - r if r > H - 1 else r)


def _mv_dr(t, npart, m, i_stride, off):
    """Overlapping DoubleRow moving AP [npart, (2,i_stride), (1,m)]."""
    ap = t[0:npart, off:off + 1].unsqueeze(1)
    fstride = t[:].ap[0][0]
    ap.ap = bass_rust.VecI64Pair([(fstride, npart), (i_stride, 2), (1, m)])
    return ap


def _build():
    nc = bacc.Bacc("TRN2", target_bir_lowering=False, debug=False,
                   num_devices=NCORES)
    xin = nc.dram_tensor("xin", [BPC, C, H, W], F32, kind="ExternalInput").ap()
    msk = nc.dram_tensor("msk", [NSTEP, BPC, 128, GQ * W], F8,
                         kind="ExternalInput").ap()
    wa0 = nc.dram_tensor("wa0", [97, 256], F8, kind="ExternalInput").ap()
    wa1 = nc.dram_tensor("wa1", [97, 256], F8, kind="ExternalInput").ap()
    wb = nc.dram_tensor("wb", [96, 256], F16, kind="ExternalInput").ap()
    w2 = nc.dram_tensor("w2", [128, 64], F16, kind="ExternalInput").ap()
    yout = nc.dram_tensor("y", [BPC, 128, GQ * W], F16, kind="ExternalOutput").ap()

    with tile.TileContext(nc) as tc, ExitStack() as ctx:
        wpool = ctx.enter_context(tc.tile_pool(name="wpool", bufs=1))
        wat0 = wpool.tile([97, 256], F8, name="wat0")
        wat1 = wpool.tile([97, 256], F8, name="wat1")
        wbt = wpool.tile([96, 256], F16, name="wbt")
        w2t = wpool.tile([128, 64], F16, name="w2t")
        nc.sync.dma_start(wat0[:], wa0)
        nc.sync.dma_start(wat1[:], wa1)
        nc.sync.dma_start(wbt[:], wb)
        nc.sync.dma_start(w2t[:], w2)
        war0 = wat0[:].rearrange("p (i n) -> p i n", i=2)  # [97, 2, 128]
        war1 = wat1[:].rearrange("p (i n) -> p i n", i=2)

        xex = wpool.tile([128, GQ * W], F16, name="xex")
        # stage tiles: 2 alternating buffers each (band parity), persistent
        # so the stage8 ones-row (bias) survives across bands.
        st16 = [wpool.tile([96, BR * W], F16, name=f"st16_{i}") for i in (0, 1)]
        st8 = [wpool.tile([97, BR * WP], F8, name=f"st8_{i}") for i in (0, 1)]
        for t in st8:
            nc.gpsimd.memset(t[96:97, :], 1.0)

        mk_pool = ctx.enter_context(tc.tile_pool(name="mk", bufs=2))
        hs_pool = ctx.enter_context(tc.tile_pool(name="hs", bufs=9))
        t_pool = ctx.enter_context(tc.tile_pool(name="tp", bufs=2))
        hp_pool = ctx.enter_context(tc.tile_pool(name="hp", bufs=5, space="PSUM"))
        dx_pool = ctx.enter_context(tc.tile_pool(name="dxp", bufs=3, space="PSUM"))

        def load_stage(s, b):
            """Stage band b's shifted x views from xex (reads pre-update
            values: call before band b-1's residual writes land)."""
            s16, s8 = st16[b % 2], st8[b % 2]
            s16r = s16[:].rearrange("p (r c) -> p r c", c=W)
            s8r = s8[:].rearrange("p (r c) -> p r c", c=WP)
            xr = xex[:].rearrange("p (g c) -> p g c", c=W)
            for gi, dy in enumerate(DYS):
                p0 = gi * 32
                # stage slot rr holds x row refl(16b+rr+dy)
                rows = [_refl(BR * b + rr + dy) for rr in range(BR)]
                # batch by phase where rows form stride-4 runs
                done = [False] * BR
                for rr0 in range(BR):
                    if done[rr0]:
                        continue
                    run = [rr0]
                    while len(run) < 4:
                        nxt = run[-1] + 4
                        if nxt < BR and rows[nxt] == rows[run[-1]] + 4:
                            run.append(nxt)
                        else:
                            break
                    for rr in run:
                        done[rr] = True
                    r0 = rows[rr0]
                    ph, g0, n = r0 % 4, r0 // 4, len(run)
                    src = xr[32 * ph:32 * ph + 32, g0:g0 + n, :]
                    end = rr0 + 4 * (n - 1) + 1
                    nc.sync.dma_start(
                        s16r[p0:p0 + 32, rr0:end:4, :], src)
                    nc.gpsimd.dma_start(
                        s8r[p0:p0 + 32, rr0:end:4, 1:W + 1], src)
            # reflect col pads for stage8 (cols 0 and 257)
            nc.vector.tensor_copy(s8r[0:96, :, 0:1], s8r[0:96, :, 2:3])
            nc.vector.tensor_copy(s8r[0:96, :, WP - 1:WP],
                                  s8r[0:96, :, WP - 3:WP - 2])
            return s16, s8

        def do_l2(nc2, item):
            """Layer 2 for one row; on the group's last row, emit the
            masked-residual chain for that group's dxp."""
            hs, j, dxp, mk, g4, g = item
            out = dxp[32 * j:32 * j + 32, 0:256]
            nc2.tensor.matmul(out, w2t[:, 0:32], hs[:, 0:256],
                              start=True, stop=False, tile_position=(0, 32 * j))
            nc2.tensor.matmul(out, w2t[:, 32:64], hs[:, 256:512],
                              start=False, stop=True, tile_position=(0, 32 * j))
            if j == 3:
                # ---- masked residual: xex += dxp * mask ----
                tt = t_pool.tile([128, 256], F32)
                nc2.vector.tensor_tensor(
                    tt[:], dxp[:, 0:256],
                    mk[:, g4 * 256:(g4 + 1) * 256], OP.mult)
                nc2.vector.tensor_tensor(
                    xex[:, g * W:(g + 1) * W], tt[:],
                    xex[:, g * W:(g + 1) * W], OP.add)

        for s in range(BPC):
            # ---- init: load x into xex (fp32 -> fp16, 4 phase DMAs) ----
            xr = xex[:].rearrange("p (g c) -> p g c", c=W)
            for ph in range(4):
                nc.gpsimd.dma_start(
                    xr[32 * ph:32 * ph + 32, :, :],
                    xin[s, :, ph:H:4, :])
            for st in range(NSTEP):
                pend = []  # rows awaiting layer 2 (software pipeline, lag 2)
                for b in range(NB):
                    if b == 0:
                        s16, s8 = load_stage(s, 0)
                        mk = mk_pool.tile([128, 1024], F8)
                        nc.sync.dma_start(mk[:], msk[st, s, :, 0:1024])
                    else:
                        s16, s8 = st16[b % 2], st8[b % 2]
                        mk = mk_nxt
                    # prefetch next band's stages + mask BEFORE this band's
                    # residual writes touch xex (halo correctness)
                    if b + 1 < NB:
                        load_stage(s, b + 1)
                        mk_nxt = mk_pool.tile([128, 1024], F8)
                        nc.sync.dma_start(
                            mk_nxt[:],
                            msk[st, s, :, (b + 1) * 1024:(b + 2) * 1024])
                    s16f = s16[:]
                    for g4 in range(4):
                        dxp = dx_pool.tile([128, 512], F32)
                        for j in range(4):
                            rr = 4 * g4 + j
                            hp = hp_pool.tile([128, 512], F32)
                            mva = _mv_dr(s8, 97, 256, 2, rr * WP)
                            mvb = s16f[0:96, rr * W:(rr + 1) * W]
                            # one accumulation group per PSUM bank: A(h0)'s
                            # start zeroes the 2KB region, A(h1) lands on
                            # pending-zero bytes, B's accumulate on top.
                            nc.tensor.matmul(hp[:, 0:256], war0[:],
                                             mva, start=True, stop=False,
                                             perf_mode=DR)
                            nc.tensor.matmul(hp[:, 256:512], war1[:],
                                             mva, start=False, stop=False,
                                             perf_mode=DR)
                            nc.tensor.matmul(hp[:, 0:256], wbt[:, 0:128],
                                             mvb, start=False, stop=False)
                            nc.tensor.matmul(hp[:, 256:512], wbt[:, 128:256],
                                             mvb, start=False, stop=True)
                            hs = hs_pool.tile([128, 512], F16)
                            if rr % 4 == 3:
                                nc.vector.tensor_relu(hs[:], hp[:])
                            else:
                                nc.scalar.activation(hs[:], hp[:], AF.Relu)
                            pend.append((hs, j, dxp, mk, g4, 4 * b + g4))
                            if len(pend) > 6:
                                do_l2(nc, pend.pop(0))
                while pend:
                    do_l2(nc, pend.pop(0))
            # ---- store sample result (raw xex layout, host unpacks) ----
            nc.sync.dma_start(yout[s], xex[:])
    nc.compile()
    return nc


_NC_CACHE = None


def _get_nc():
    global _NC_CACHE
    if _NC_CACHE is None:
        _NC_CACHE = _build()
    return _NC_CACHE


def _make_in_maps(x, f1, f2, w1, b1, w2, stoch):
    F8N = ml_dtypes.float8_e4m3
    f1 = np.asarray(f1, np.float64)[:, :, 0, :]   # [3,3,32]
    f2 = np.asarray(f2, np.float64)[:, :, 0, :]
    w1 = np.asarray(w1, np.float64)               # [96,256]
    b1 = np.asarray(b1, np.float64)               # [256]
    w2m = np.asarray(w2, np.float64).copy()       # [256,32]
    w2m[:, :IMG] = 0.0

    # W_eff[dy,dx][c,:] = f1*w1[32:64] + f2*w1[64:96] (+ w1[0:32] center)
    weff = (f1[:, :, :, None] * w1[None, None, 32:64, :]
            + f2[:, :, :, None] * w1[None, None, 64:96, :])   # [3,3,32,256]
    weff[1, 1] += w1[0:32, :]

    # chunk A (fp8 DR): taps (dy, dx in {0,2}); p = dyg*32+c, dyg=(0,-1,+1)
    dy_rows = (1, 0, 2)  # weff dy index per partition group
    wa = np.zeros((97, 2, 256), np.float64)
    for gi, dyi in enumerate(dy_rows):
        for i, dxi in enumerate((0, 2)):
            wa[gi * 32:(gi + 1) * 32, i, :] = weff[dyi, dxi]
    wa[96, 0, :] = b1
    wa8 = wa.astype(F8N)
    wa80 = np.ascontiguousarray(wa8[:, :, 0:128]).reshape(97, 256)
    wa81 = np.ascontiguousarray(wa8[:, :, 128:256]).reshape(97, 256)
    # chunk B (fp16): center column taps (dy, dx=1)
    wbm = np.zeros((96, 256), np.float64)
    for gi, dyi in enumerate(dy_rows):
        wbm[gi * 32:(gi + 1) * 32, :] = weff[dyi, 1]
    wb16 = wbm.astype(np.float16)
    # layer 2 (fp16): [p, h*32+n] = w2m[h*128+p, n]
    w2h = np.concatenate([w2m[0:128, :], w2m[128:256, :]],
                         axis=1).astype(np.float16)

    x = np.asarray(x, np.float32)
    stoch = np.asarray(stoch, np.float32)
    in_maps = []
    for i in range(NCORES):
        xi = np.ascontiguousarray(
            np.transpose(x[i * BPC:(i + 1) * BPC], (0, 3, 1, 2)))  # [2,32,H,W]
        m = (stoch[:, i * BPC:(i + 1) * BPC, :, :, 0] > 0.5)  # [4,2,H,W]
        # [st,s,128,GQ*W]: partition 32*ph+c <- mask row 4g+ph
        mm = m.reshape(NSTEP, BPC, GQ, 4, W).transpose(0, 1, 3, 2, 4)
        mm = mm.reshape(NSTEP, BPC, 4, GQ * W)
        mrep = np.repeat(mm, 32, axis=2).astype(F8N)
        in_maps.append({"xin": xi, "msk": mrep, "wa0": wa80, "wa1": wa81,
                        "wb": wb16, "w2": w2h})
    return in_maps


def kernel(x, f1, f2, w1, b1, w2, stoch, steps):
    assert int(steps) == NSTEP, f"kernel compiled for {NSTEP} steps, got {steps}"
    nc = _get_nc()
    in_maps = _make_in_maps(x, f1, f2, w1, b1, w2, stoch)
    res = run_bass_kernel_spmd(nc, in_maps, core_ids=list(range(NCORES)))
    outs = []
    for i in range(NCORES):
        yi = np.asarray(res.results[i]["y"], np.float32)  # [2,128,GQ*W]
        # partition 32*ph+c, free g*W+col -> x[c, 4g+ph, col]
        yi = yi.reshape(BPC, 4, 32, GQ, W).transpose(0, 2, 3, 1, 4)
        outs.append(yi.reshape(BPC, C, H, W).transpose(0, 2, 3, 1))
    return np.ascontiguousarray(np.concatenate(outs, axis=0)).astype(np.float32)


# revision 13
# speedup vs baseline: 1.4803x; 1.2930x over previous
"""Trainium2 Bass kernel for nn_BasicNCAModel (neural cellular automaton).

Model (per step, 4 steps):
  y = concat([x, dwconv3x3(x, f1), dwconv3x3(x, f2)])   (reflect pad)
  dx = relu(y @ w1 + b1) @ w2
  x  = x + dx * (stoch > 0.5) * ch_mask

Strategy (v2, fp8-DoubleRow hybrid):
  - Pure data parallel: batch 16 -> 2 samples per core, samples sequential.
  - x lives RESIDENT in SBUF in fp16 ("xex", layout [128, (H/4)*W]:
    partition = 32*(row%4) + channel), updated in place each step. No DRAM
    round-trips between steps; DRAM I/O is the initial load, the per-step
    masks, and the final store.
  - The depthwise convs + first dense layer fold into a 3x3 conv with
    effective weights W_eff[dy,dx] (the baseline trick). Per output row
    (256 px) the conv splits into:
      * chunk A (fp8 e4m3, DoubleRow): the 6 off-center-column taps
        (dy in {-1,0,1} x dx in {0,2}) packed 2-per-PE-cell via an
        overlapping moving AP [97,(2,stride 2),(1,256)], + bias via a
        ones partition. 1 matmul per N-half.
      * chunk B (fp16): the 3 center-column taps (dx=1) - these carry the
        dominant identity path w1[0:32], so they stay in fp16 for accuracy.
        1 matmul per N-half.
    Layer 2 runs in fp16 (2 matmuls, K=128 each). Total 6 matmuls / row
    vs 8 fp16-only: matmul cost on TRN2 is out_rows * 0.42ns regardless
    of K<=128 (2*128 with DoubleRow), so this is a 1.33x tensor win with
    fp8 confined to the small off-center taps (max rel err ~1.3e-2 vs
    2e-2 budget, numpy-simulated).
  - Per 16-row band, shifted views of x are STAGED from xex:
    stage16 [96, 16*256] fp16 (3 dy-shifted copies, HWDGE) for chunk B,
    stage8 [97, 16*258] fp8 (SWDGE cast, + reflect col pads) for chunk A.
    Stage loads for band b+1 are emitted before band b's residual writes,
    so the in-place xex update never corrupts halo reads.
  - Residual: dx masked with host-precomputed {0,1} fp8 masks (replicated
    to the [128, .] layout), added to xex in fp32->fp16 on the vector
    engine, 128 partitions wide (4 rows per op).
"""

import numpy as np
import ml_dtypes
from contextlib import ExitStack

import concourse.bacc as bacc
import concourse.tile as tile
from concourse import mybir
from concourse.bass_utils import run_bass_kernel_spmd
import bass_rust

F32 = mybir.dt.float32
F16 = mybir.dt.float16
F8 = mybir.dt.float8e4
AF = mybir.ActivationFunctionType
OP = mybir.AluOpType
DR = mybir.MatmulPerfMode.DoubleRow

B, C, H, W = 16, 32, 256, 256
IMG = 3
NCORES = 8
BPC = B // NCORES          # samples per core = 2
NSTEP = 4
WP = W + 2                 # stage8 row pitch (reflect col pads)
BR = 16                    # band rows
NB = H // BR               # bands = 16
GQ = H // 4                # 4-row groups per image = 64
DYS = (0, -1, 1)           # dy per stage partition group


def _refl(r):
    return -r if r < 0 else (2 * (H - 1) - r if r > H - 1 else r)


def _mv_dr(t, npart, m, i_stride, off):
    """Overlapping DoubleRow moving AP [npart, (2,i_stride), (1,m)]."""
    ap = t[0:npart, off:off + 1].unsqueeze(1)
    fstride = t[:].ap[0][0]
    ap.ap = bass_rust.VecI64Pair([(fstride, npart), (i_stride, 2), (1, m)])
    return ap


def _build():
    nc = bacc.Bacc("TRN2", target_bir_lowering=False, debug=False,
                   num_devices=NCORES)
    xin = nc.dram_tensor("xin", [BPC, C, H, W], F32, kind="ExternalInput").ap()
    msk = nc.dram_tensor("msk", [NSTEP, BPC, 128, GQ * W], F8,
                         kind="ExternalInput").ap()
    wa0 = nc.dram_tensor("wa0", [97, 256], F8, kind="ExternalInput").ap()
    wa1 = nc.dram_tensor("wa1", [97, 256], F8, kind="ExternalInput").ap()
    wb = nc.dram_tensor("wb", [96, 256], F16, kind="ExternalInput").ap()
    w2 = nc.dram_tensor("w2", [128, 64], F16, kind="ExternalInput").ap()
    yout = nc.dram_tensor("y", [BPC, 128, GQ * W], F16, kind="ExternalOutput").ap()

    with tile.TileContext(nc) as tc, ExitStack() as ctx:
        wpool = ctx.enter_context(tc.tile_pool(name="wpool", bufs=1))
        wat0 = wpool.tile([97, 256], F8, name="wat0")
        wat1 = wpool.tile([97, 256], F8, name="wat1")
        wbt = wpool.tile([96, 256], F16, name="wbt")
        w2t = wpool.tile([128, 64], F16, name="w2t")
        nc.sync.dma_start(wat0[:], wa0)
        nc.sync.dma_start(wat1[:], wa1)
        nc.sync.dma_start(wbt[:], wb)
        nc.sync.dma_start(w2t[:], w2)
        war0 = wat0[:].rearrange("p (i n) -> p i n", i=2)  # [97, 2, 128]
        war1 = wat1[:].rearrange("p (i n) -> p i n", i=2)

        xex = wpool.tile([128, GQ * W], F16, name="xex")
        # stage tiles: 2 alternating buffers each (band parity), persistent
        # so the stage8 ones-row (bias) survives across bands.
        st16 = [wpool.tile([96, BR * W], F16, name=f"st16_{i}") for i in (0, 1)]
        st8 = [wpool.tile([97, BR * WP], F8, name=f"st8_{i}") for i in (0, 1)]
        for t in st8:
            nc.gpsimd.memset(t[96:97, :], 1.0)

        mk_pool = ctx.enter_context(tc.tile_pool(name="mk", bufs=2))
        hs_pool = ctx.enter_context(tc.tile_pool(name="hs", bufs=8))
        t_pool = ctx.enter_context(tc.tile_pool(name="tp", bufs=2))
        hp_pool = ctx.enter_context(tc.tile_pool(name="hp", bufs=4, space="PSUM"))
        dx_pool = ctx.enter_context(tc.tile_pool(name="dxp", bufs=4, space="PSUM"))

        def load_stage(s, b):
            """Stage band b's shifted x views from xex (reads pre-update
            values: call before band b-1's residual writes land)."""
            s16, s8 = st16[b % 2], st8[b % 2]
            s16r = s16[:].rearrange("p (r c) -> p r c", c=W)
            s8r = s8[:].rearrange("p (r c) -> p r c", c=WP)
            xr = xex[:].rearrange("p (g c) -> p g c", c=W)
            for gi, dy in enumerate(DYS):
                p0 = gi * 32
                # stage slot rr holds x row refl(16b+rr+dy)
                rows = [_refl(BR * b + rr + dy) for rr in range(BR)]
                # batch by phase where rows form stride-4 runs
                done = [False] * BR
                for rr0 in range(BR):
                    if done[rr0]:
                        continue
                    run = [rr0]
                    while len(run) < 4:
                        nxt = run[-1] + 4
                        if nxt < BR and rows[nxt] == rows[run[-1]] + 4:
                            run.append(nxt)
                        else:
                            break
                    for rr in run:
                        done[rr] = True
                    r0 = rows[rr0]
                    ph, g0, n = r0 % 4, r0 // 4, len(run)
                    src = xr[32 * ph:32 * ph + 32, g0:g0 + n, :]
                    end = rr0 + 4 * (n - 1) + 1
                    nc.sync.dma_start(
                        s16r[p0:p0 + 32, rr0:end:4, :], src)
                    nc.gpsimd.dma_start(
                        s8r[p0:p0 + 32, rr0:end:4, 1:W + 1], src)
            # reflect col pads for stage8 (cols 0 and 257)
            nc.vector.tensor_copy(s8r[0:96, :, 0:1], s8r[0:96, :, 2:3])
            nc.vector.tensor_copy(s8r[0:96, :, WP - 1:WP],
                                  s8r[0:96, :, WP - 3:WP - 2])
            return s16, s8

        def do_l2(nc2, item):
            """Layer 2 for one row; on the group's last row, emit the
            masked-residual chain for that group's dxp."""
            hs, j, dxp, mk, g4, g = item
            out = dxp[32 * j:32 * j + 32, 0:256]
            nc2.tensor.matmul(out, w2t[:, 0:32], hs[:, 0:256],
                              start=True, stop=False, tile_position=(0, 32 * j))
            nc2.tensor.matmul(out, w2t[:, 32:64], hs[:, 256:512],
                              start=False, stop=True, tile_position=(0, 32 * j))
            if j == 3:
                # ---- masked residual: xex += dxp * mask ----
                tt = t_pool.tile([128, 256], F32)
                nc2.vector.tensor_tensor(
                    tt[:], dxp[:, 0:256],
                    mk[:, g4 * 256:(g4 + 1) * 256], OP.mult)
                nc2.vector.tensor_tensor(
                    xex[:, g * W:(g + 1) * W], tt[:],
                    xex[:, g * W:(g + 1) * W], OP.add)

        for s in range(BPC):
            # ---- init: load x into xex (fp32 -> fp16, 4 phase DMAs) ----
            xr = xex[:].rearrange("p (g c) -> p g c", c=W)
            for ph in range(4):
                nc.gpsimd.dma_start(
                    xr[32 * ph:32 * ph + 32, :, :],
                    xin[s, :, ph:H:4, :])
            for st in range(NSTEP):
                pend = []  # rows awaiting layer 2 (software pipeline, lag 2)
                for b in range(NB):
                    if b == 0:
                        s16, s8 = load_stage(s, 0)
                        mk = mk_pool.tile([128, 1024], F8)
                        nc.sync.dma_start(mk[:], msk[st, s, :, 0:1024])
                    else:
                        s16, s8 = st16[b % 2], st8[b % 2]
                        mk = mk_nxt
                    # prefetch next band's stages + mask BEFORE this band's
                    # residual writes touch xex (halo correctness)
                    if b + 1 < NB:
                        load_stage(s, b + 1)
                        mk_nxt = mk_pool.tile([128, 1024], F8)
                        nc.sync.dma_start(
                            mk_nxt[:],
                            msk[st, s, :, (b + 1) * 1024:(b + 2) * 1024])
                    s16f = s16[:]
                    for g4 in range(4):
                        dxp = dx_pool.tile([128, 512], F32)
                        for j in range(4):
                            rr = 4 * g4 + j
                            hp = hp_pool.tile([128, 512], F32)
                            mva = _mv_dr(s8, 97, 256, 2, rr * WP)
                            mvb = s16f[0:96, rr * W:(rr + 1) * W]
                            # one accumulation group per PSUM bank: A(h0)'s
                            # start zeroes the 2KB region, A(h1) lands on
                            # pending-zero bytes, B's accumulate on top.
                            nc.tensor.matmul(hp[:, 0:256], war0[:],
                                             mva, start=True, stop=False,
                                             perf_mode=DR)
                            nc.tensor.matmul(hp[:, 256:512], war1[:],
                                             mva, start=False, stop=False,
                                             perf_mode=DR)
                            nc.tensor.matmul(hp[:, 0:256], wbt[:, 0:128],
                                             mvb, start=False, stop=False)
                            nc.tensor.matmul(hp[:, 256:512], wbt[:, 128:256],
                                             mvb, start=False, stop=True)
                            hs = hs_pool.tile([128, 512], F16)
                            if rr % 4 == 3:
                                nc.vector.tensor_relu(hs[:], hp[:])
                            else:
                                nc.scalar.activation(hs[:], hp[:], AF.Relu)
                            pend.append((hs, j, dxp, mk, g4, 4 * b + g4))
                            if len(pend) > 6:
                                do_l2(nc, pend.pop(0))
                while pend:
                    do_l2(nc, pend.pop(0))
            # ---- store sample result (raw xex layout, host unpacks) ----
            nc.sync.dma_start(yout[s], xex[:])
    nc.compile()
    return nc


_NC_CACHE = None


def _get_nc():
    global _NC_CACHE
    if _NC_CACHE is None:
        _NC_CACHE = _build()
    return _NC_CACHE


def _make_in_maps(x, f1, f2, w1, b1, w2, stoch):
    F8N = ml_dtypes.float8_e4m3
    f1 = np.asarray(f1, np.float64)[:, :, 0, :]   # [3,3,32]
    f2 = np.asarray(f2, np.float64)[:, :, 0, :]
    w1 = np.asarray(w1, np.float64)               # [96,256]
    b1 = np.asarray(b1, np.float64)               # [256]
    w2m = np.asarray(w2, np.float64).copy()       # [256,32]
    w2m[:, :IMG] = 0.0

    # W_eff[dy,dx][c,:] = f1*w1[32:64] + f2*w1[64:96] (+ w1[0:32] center)
    weff = (f1[:, :, :, None] * w1[None, None, 32:64, :]
            + f2[:, :, :, None] * w1[None, None, 64:96, :])   # [3,3,32,256]
    weff[1, 1] += w1[0:32, :]

    # chunk A (fp8 DR): taps (dy, dx in {0,2}); p = dyg*32+c, dyg=(0,-1,+1)
    dy_rows = (1, 0, 2)  # weff dy index per partition group
    wa = np.zeros((97, 2, 256), np.float64)
    for gi, dyi in enumerate(dy_rows):
        for i, dxi in enumerate((0, 2)):
            wa[gi * 32:(gi + 1) * 32, i, :] = weff[dyi, dxi]
    wa[96, 0, :] = b1
    wa8 = wa.astype(F8N)
    wa80 = np.ascontiguousarray(wa8[:, :, 0:128]).reshape(97, 256)
    wa81 = np.ascontiguousarray(wa8[:, :, 128:256]).reshape(97, 256)
    # chunk B (fp16): center column taps (dy, dx=1)
    wbm = np.zeros((96, 256), np.float64)
    for gi, dyi in enumerate(dy_rows):
        wbm[gi * 32:(gi + 1) * 32, :] = weff[dyi, 1]
    wb16 = wbm.astype(np.float16)
    # layer 2 (fp16): [p, h*32+n] = w2m[h*128+p, n]
    w2h = np.concatenate([w2m[0:128, :], w2m[128:256, :]],
                         axis=1).astype(np.float16)

    x = np.asarray(x, np.float32)
    stoch = np.asarray(stoch, np.float32)
    in_maps = []
    for i in range(NCORES):
        xi = np.ascontiguousarray(
            np.transpose(x[i * BPC:(i + 1) * BPC], (0, 3, 1, 2)))  # [2,32,H,W]
        m = (stoch[:, i * BPC:(i + 1) * BPC, :, :, 0] > 0.5)  # [4,2,H,W]
        # [st,s,128,GQ*W]: partition 32*ph+c <- mask row 4g+ph
        mm = m.reshape(NSTEP, BPC, GQ, 4, W).transpose(0, 1, 3, 2, 4)
        mm = mm.reshape(NSTEP, BPC, 4, GQ * W)
        mrep = np.repeat(mm, 32, axis=2).astype(F8N)
        in_maps.append({"xin": xi, "msk": mrep, "wa0": wa80, "wa1": wa81,
                        "wb": wb16, "w2": w2h})
    return in_maps


def kernel(x, f1, f2, w1, b1, w2, stoch, steps):
    assert int(steps) == NSTEP, f"kernel compiled for {NSTEP} steps, got {steps}"
    nc = _get_nc()
    in_maps = _make_in_maps(x, f1, f2, w1, b1, w2, stoch)
    res = run_bass_kernel_spmd(nc, in_maps, core_ids=list(range(NCORES)))
    outs = []
    for i in range(NCORES):
        yi = np.asarray(res.results[i]["y"], np.float32)  # [2,128,GQ*W]
        # partition 32*ph+c, free g*W+col -> x[c, 4g+ph, col]
        yi = yi.reshape(BPC, 4, 32, GQ, W).transpose(0, 2, 3, 1, 4)
        outs.append(yi.reshape(BPC, C, H, W).transpose(0, 2, 3, 1))
    return np.ascontiguousarray(np.concatenate(outs, axis=0)).astype(np.float32)
